# revision 29
# baseline (speedup 1.0000x reference)
"""Causal single-head attention on 8 trn2 NeuronCores — fp8 DoubleRow + bf16 col.

Problem: x [4, 2048, 1024] f32; Wq/Wk/Wv [1024, 1024] f32.
  q,k,v = x@W*; scores = q@k^T (causal masked, scaled 1/sqrt(1024));
  out = softmax(scores) @ v.

Sharding: 8 cores = 4 batches x 2 query-parities. Core c: batch c//2,
parity h=c%2 owns the 256-row query cols {0,3,4,7} (h=0) or {1,2,5,6}
(h=1) -- both parities see causal extents {1,2,3,4} (in 512-key cols),
so one SPMD program fits all cores exactly; per-core causal masks ride
in as data and cover the <=256 keys of block padding per col.

Precision: all matmuls fp8(e4m3) with MatmulPerfMode.DoubleRow
(operands [128,2,N] = two 128-row contraction tiles per instruction,
fp32 PSUM accumulate, ~2x PE throughput), EXCEPT the value path of the
extent-1 query col (queries < 512): those are the only rows with
concentrated attention, where x/Wv quantization noise lands at full
output magnitude (fp8 there measured 4.5e-2 rel err); they use bf16
x/exp/Wv instead (measured 1.3e-2). Scale management (powers of 2,
exact):
  - Wq/Wk/Wv prescaled x32 on host (raw sigma~0.018 sits in fp8's
    subnormal range below 2^-6).  scores' = 1024*s; exp arg scale 2^-15.
  - fp8 TT copied PSUM->SBUF with per-col scale TTS[jc] (keeps empirical
    |TT| max ~240-500 under fp8 max 240; larger scale on short cols so
    1/rowsum-amplified quantization noise stays small).
  - rowsum ones-vector = 32*TTS[jc] (col jc of a [P,4] fp8 tile; 32.0
    for the bf16 col) makes out = po * (1/rowsum_tile) exact.

Col order by extent (2,1,4,3); cols that share causal key-blocks are
fused so their scores run as one 512-query-wide matmul group (hides the
256-cycle DoubleRow LDWEIGHTS behind 512-row matmuls and halves the
instruction count there): cols 0+1 share kbs 0..3 (one group, two exp
reads -- fp8 half for col 0, bf16 half for col 1), cols 2+3 share kbs
0..11 (joint scores/exp/rowsum/TT in [P,512] PSUM; col2-only kbs 12..15
accumulate into the low half).  Each col keeps its own EXPS slots, so
no WAR chain serializes the kernel tail into a HAM re-throttle.
Startup and tail split DMA descriptor generation (~600ns each,
serialized per engine) across the sync and scalar HWDGE queues.

kernel() is self-contained: shards on host, runs via run_bass_kernel_spmd
on cores 0-7, reassembles the full [4, 2048, 1024] output.
"""

import numpy as np
import ml_dtypes
from contextlib import ExitStack

import concourse.bass as bass
import concourse.mybir as mybir
import concourse.tile as tile
from concourse import bacc
from concourse.bass_utils import run_bass_kernel_spmd

P = 128
D = 1024          # d_in == d_out
NSEQ = 2048
NCOL = 512        # projection moving width / key-col unit
QW = 256          # query col width in phase 2
DB = D // P       # 8 d blocks
EB = D // P       # 8 e blocks
EXT = (2, 1, 4, 3)           # causal extent per local q col, in 512-key cols
BCOL = 1                     # the bf16 value-path col (extent 1)
QCOLS = {0: (3, 0, 7, 4), 1: (2, 1, 6, 5)}  # parity -> global 256-q-cols
# per-col fp8 TT copy scale, by extent {1:1/2, 2:1/4, 3:1/8, 4:1/8}
TTS = tuple({1: 0.5, 2: 0.25, 3: 0.125, 4: 0.125}[e] for e in EXT)

_f32 = mybir.dt.float32
_f8 = mybir.dt.float8e4
_bf = mybir.dt.bfloat16
_DR = mybir.MatmulPerfMode.DoubleRow
F8 = ml_dtypes.float8_e4m3
BF = ml_dtypes.bfloat16

_BUILD_CACHE = {}


def _build():
    if "nc" in _BUILD_CACHE:
        return _BUILD_CACHE["nc"]

    nc = bacc.Bacc("TRN2", target_bir_lowering=False, debug=False, num_devices=8)
    # host-pretiled activations; every DMA reads contiguous records
    # xt[p, ic, db, n]   = x^T[db*128+p, ic*512+n]        fp8
    # xtq[p, jc, db, n]  = gathered-q x^T[db*128+p, .]    fp8
    # xk[p, db, kb, m]   = x[kb*128+p, db*128+m]          fp8 (resident)
    # xkb                = same, keys 0..511 only         bf16
    xt = nc.dram_tensor("xt", [P, 4, DB, NCOL], _f8, kind="ExternalInput").ap()
    xtq = nc.dram_tensor("xtq", [P, 2, DB, NCOL], _f8, kind="ExternalInput").ap()
    xk = nc.dram_tensor("xk", [P, DB, 16, P], _f8, kind="ExternalInput").ap()
    xkb = nc.dram_tensor("xkb", [P, DB, 4, P], _bf, kind="ExternalInput").ap()
    # host-prechunked x32-scaled weights
    wq = nc.dram_tensor("wq", [P, EB, DB, P], _f8, kind="ExternalInput").ap()
    wk = nc.dram_tensor("wk", [P, EB, DB, P], _f8, kind="ExternalInput").ap()
    wv = nc.dram_tensor("wv", [P, DB, 2, NCOL], _f8, kind="ExternalInput").ap()
    wvb = nc.dram_tensor("wvb", [P, DB, 2, NCOL], _bf, kind="ExternalInput").ap()
    msk = nc.dram_tensor("msk", [P, 4, 4, QW], _f8, kind="ExternalInput").ap()
    mskb = nc.dram_tensor("mskb", [P, 4, QW], _bf, kind="ExternalInput").ap()
    onesd = nc.dram_tensor("ones", [P, 2, 4], _f8, kind="ExternalInput").ap()
    onesb = nc.dram_tensor("onesb", [P, 1], _bf, kind="ExternalInput").ap()
    out = nc.dram_tensor("out", [1024, D], _f32, kind="ExternalOutput").ap()

    escale = float(2.0 ** -15)   # exp argument scale: 1/(32*sqrt(D)*32*32)

    with tile.TileContext(nc) as tc, ExitStack() as ctx:
        pers = ctx.enter_context(tc.tile_pool(name="pers", bufs=1))
        KT = pers.tile([P, EB, 4, NCOL], _f8)        # 16 KB/part
        QT = pers.tile([P, EB, 2, NCOL], _f8)        # 8
        XK = pers.tile([P, DB, 16, P], _f8)          # 16
        ONES = pers.tile([P, 2, 4], _f8)

        # ---- phase 1: KT / QT projections ----
        with ExitStack() as p1:
            wpool = p1.enter_context(tc.tile_pool(name="wpool", bufs=1))
            WK = wpool.tile([P, EB, DB, P], _f8)     # 8
            WQ = wpool.tile([P, EB, DB, P], _f8)     # 8
            xcol = p1.enter_context(tc.tile_pool(name="xcol", bufs=3))
            ps_proj = p1.enter_context(tc.tile_pool(name="ps_proj", bufs=4, space="PSUM"))

            xcs = [xcol.tile([P, DB, NCOL], _f8, tag="xc", name=f"xc{i}") for i in range(4)]
            # startup: descriptor generation serializes at ~600ns/DMA on the
            # sync engine, so the first two DMAs are exactly what matmul #1
            # needs (WK eb0 pair 0 + xc0 pair 0, 160KB); the rest pace behind
            nc.scalar.dma_start(WK[:, 0, 0:2, :], wk[:, 0, 0:2, :])
            nc.sync.dma_start(xcs[0][:, 0:2, :], xt[:, 0, 0:2, :])
            nc.scalar.dma_start(WK[:, 0, 2:, :], wk[:, 0, 2:, :])
            nc.sync.dma_start(xcs[0][:, 2:4, :], xt[:, 0, 2:4, :])
            nc.sync.dma_start(xcs[0][:, 4:8, :], xt[:, 0, 4:8, :])
            for eb in range(1, 4):
                nc.scalar.dma_start(WK[:, eb, :, :], wk[:, eb, :, :])
            nc.sync.dma_start(xcs[1][:], xt[:, 1, :, :])
            for eb in range(4, EB):
                nc.scalar.dma_start(WK[:, eb, :, :], wk[:, eb, :, :])
            nc.sync.dma_start(ONES[:], onesd)

            for ic in range(4):
                xc = xcs[ic]
                for eb in range(EB):
                    if eb == 5 and ic + 2 < 4:
                        nc.sync.dma_start(xcs[ic + 2][:], xt[:, ic + 2, :, :])
                    if eb == 2 and ic >= 2:
                        for wqe in range(4 * (ic - 2), 4 * (ic - 1)):
                            nc.sync.dma_start(WQ[:, wqe, :, :], wq[:, wqe, :, :])
                    ps = ps_proj.tile([P, NCOL], _f32)
                    for t in range(4):
                        nc.tensor.matmul(ps[:], WK[:, eb, 2 * t:2 * t + 2, :],
                                         xc[:, 2 * t:2 * t + 2, :],
                                         start=(t == 0), stop=(t == 3),
                                         perf_mode=_DR)
                    # ACT copy: keeps DVE free, ACT is otherwise idle here
                    nc.scalar.copy(KT[:, eb, ic, :], ps[:])
            # phase-2 bulk loads queued behind the phase-1 stream
            xqs = []
            for jc in range(2):
                xq = xcol.tile([P, DB, NCOL], _f8, tag="xc", name=f"xq{jc}")
                nc.sync.dma_start(xq[:], xtq[:, jc, :, :])
                xqs.append(xq)
            for db in range(DB):
                nc.sync.dma_start(XK[:, db, :, :], xk[:, db, :, :])
            for jc in range(2):
                xq = xqs[jc]
                for eb in range(EB):
                    ps = ps_proj.tile([P, NCOL], _f32)
                    for t in range(4):
                        nc.tensor.matmul(ps[:], WQ[:, eb, 2 * t:2 * t + 2, :],
                                         xq[:, 2 * t:2 * t + 2, :],
                                         start=(t == 0), stop=(t == 3),
                                         perf_mode=_DR)
                    nc.vector.tensor_copy(QT[:, eb, jc, :], ps[:])

        # ---- phase 2: attention, per 256-wide local q col ----
        with ExitStack() as p2:
            p2sb = p2.enter_context(tc.tile_pool(name="p2sb", bufs=1))
            WV = p2sb.tile([P, DB, 2, NCOL], _f8)        # 8
            EXPS = p2sb.tile([P, 8, QW], _f8)            # col0 (ext 2)
            EXPS2 = p2sb.tile([P, 16, 2, QW], _f8)       # cols 2+3 joint
            TT2 = p2sb.tile([P, DB, 2, QW], _f8)
            TT = p2sb.tile([P, DB, QW], _f8)             # 2
            XKB = p2sb.tile([P, DB, 4, P], _bf)          # 8
            WVB = p2sb.tile([P, DB, 2, NCOL], _bf)       # 16
            EXPSB = p2sb.tile([P, 4, P], _bf)            # bcol qb0 (bf16)
            TTB = p2sb.tile([P, DB, P], _bf)
            EXPS1 = p2sb.tile([P, 4, P], _f8)            # bcol qb1 (fp8)
            TT1 = p2sb.tile([P, DB, P], _f8)
            ONESB = p2sb.tile([P, 1], _bf)
            ps_sc = p2.enter_context(tc.tile_pool(name="ps_sc", bufs=3, space="PSUM"))
            ps_rs = p2.enter_context(tc.tile_pool(name="ps_rs", bufs=1, space="PSUM"))
            ps_tt = p2.enter_context(tc.tile_pool(name="ps_tt", bufs=2, space="PSUM"))
            ps_out = p2.enter_context(tc.tile_pool(name="ps_out", bufs=2, space="PSUM"))
            mpool = p2.enter_context(tc.tile_pool(name="mpool", bufs=5))
            spool = p2.enter_context(tc.tile_pool(name="spool", bufs=2))
            dpool = p2.enter_context(tc.tile_pool(name="dram", bufs=4, space="DRAM"))
            opool = p2.enter_context(tc.tile_pool(name="opool", bufs=4))

            # phase-2 bulk loads in need order: XKB/WVB feed the bcol at
            # ~55us (WVB split across two queues), masks next, WV (joint
            # block out, ~110us) last
            nc.sync.dma_start(XKB[:], xkb)
            nc.sync.dma_start(WVB[:, :, 0, :], wvb[:, :, 0, :])
            nc.sync.dma_start(WVB[:, :, 1, :], wvb[:, :, 1, :])
            mt0 = mpool.tile([P, 4, QW], _f8, tag="mt0")
            nc.sync.dma_start(mt0[:], msk[:, 0, :, :])
            mtb = mpool.tile([P, 4, P], _bf, tag="mtb")
            nc.sync.dma_start(mtb[:], mskb[:, :, 0:P])
            mt1 = mpool.tile([P, 4, QW], _f8, tag="mt1")
            nc.sync.dma_start(mt1[:], msk[:, BCOL, :, :])
            nc.sync.dma_start(ONESB[:], onesb)
            mt2 = mpool.tile([P, 4, QW], _f8, tag="mt2")
            nc.sync.dma_start(mt2[:], msk[:, 2, :, :])
            mt3 = mpool.tile([P, 4, QW], _f8, tag="mt3")
            nc.sync.dma_start(mt3[:], msk[:, 3, :, :])
            nc.sync.dma_start(WV[:], wv)

            qv = QT.rearrange("p eb c n -> p eb (c n)")
            for jc in range(4):
                Kb = 4 * EXT[jc]     # kn 128-blocks this col
                qs = jc * QW
                if jc == BCOL:
                    # --- extent-1 col, value path split by query half:
                    # qb0 (local q 0..127, the concentrated short rows) bf16;
                    # qb1 (>=129 keys, diffuse) fp8 DoubleRow.  Exps came
                    # from col 0's shared score groups. ---
                    nc.vector.tensor_mul(EXPSB[:], EXPSB[:], mtb[:])
                    nc.vector.tensor_mul(EXPS1[:], EXPS1[:], mt1[:, :, P:])
                    rs = ps_rs.tile([1, 2 * QW], _f32)
                    for kb in range(Kb):
                        nc.tensor.matmul(rs[0:1, 0:P], ONESB[:, :],
                                         EXPSB[:, kb, :],
                                         start=(kb == 0), stop=(kb == Kb - 1))
                    for kb in range(Kb):
                        nc.tensor.matmul(rs[0:1, P:QW], ONES[:, 0, BCOL:BCOL + 1],
                                         EXPS1[:, kb, :],
                                         start=(kb == 0), stop=(kb == Kb - 1))
                    rs1 = spool.tile([1, QW], _f32, tag="rs1")
                    nc.vector.tensor_copy(rs1[0:1, :], rs[0:1, 0:QW])
                    rsd = dpool.tile([1, QW], _f32)
                    nc.sync.dma_start(rsd[:], rs1[0:1, :])
                    rst = spool.tile([P, 2], _f32, tag="rst")
                    nc.sync.dma_start(
                        rst[:], rsd.rearrange("o (q p) -> (o p) q", p=P, q=2))
                    rcp = spool.tile([P, 2], _f32, tag="rcp")
                    nc.vector.reciprocal(rcp[:], rst[:])
                    for db in range(DB):
                        pst = ps_tt.tile([P, 2 * QW], _f32)
                        for kb in range(Kb):
                            nc.tensor.matmul(pst[:, 0:P], XKB[:, db, kb, :],
                                             EXPSB[:, kb, :],
                                             start=(kb == 0), stop=(kb == Kb - 1))
                        for i in range(Kb // 2):
                            nc.tensor.matmul(pst[:, P:QW], XK[:, db, 2 * i:2 * i + 2, :],
                                             EXPS1[:, 2 * i:2 * i + 2, :],
                                             start=(i == 0), stop=(i == Kb // 2 - 1),
                                             perf_mode=_DR)
                        nc.vector.tensor_copy(TTB[:, db, :], pst[:, 0:P])
                        nc.vector.tensor_scalar_mul(TT1[:, db, :], pst[:, P:QW],
                                                    TTS[BCOL])
                    for ec in range(2):
                        po = ps_out.tile([P, NCOL], _f32, tag="po")
                        for db in range(DB):
                            nc.tensor.matmul(po[:], TTB[:, db, :],
                                             WVB[:, db, ec, :],
                                             start=(db == 0), stop=(db == DB - 1))
                        ot = opool.tile([P, NCOL], _f32, tag="ot")
                        nc.vector.tensor_scalar_mul(ot[:], po[:], rcp[:, 0:1])
                        nc.sync.dma_start(
                            out[qs: qs + P, ec * NCOL:(ec + 1) * NCOL], ot[:])
                        po1 = ps_out.tile([P, NCOL], _f32, tag="po")
                        for t in range(4):
                            nc.tensor.matmul(po1[:], TT1[:, 2 * t:2 * t + 2, :],
                                             WV[:, 2 * t:2 * t + 2, ec, :],
                                             start=(t == 0), stop=(t == 3),
                                             perf_mode=_DR)
                        ot1 = opool.tile([P, NCOL], _f32, tag="ot")
                        nc.vector.tensor_scalar_mul(ot1[:], po1[:], rcp[:, 1:2])
                        nc.sync.dma_start(
                            out[qs + P: qs + QW, ec * NCOL:(ec + 1) * NCOL], ot1[:])
                    continue
                if jc >= 2:
                    continue   # cols 2+3 handled jointly below
                # --- col 0 (ext 2) fp8 DoubleRow path; kbs 0..3 are shared
                # with col 1 (qv cols 0..511 contiguous): one 512-wide matmul
                # group, then two exp reads (fp8 half for col 0, bf16 half
                # for col 1's value path) ---
                mt = mt0
                for kb in range(Kb):
                    ps = ps_sc.tile([P, 2 * QW], _f32)
                    ic, off = kb // 4, (kb % 4) * P
                    shared = kb < 4
                    w = 2 * QW if shared else QW
                    for t in range(4):
                        nc.tensor.matmul(ps[:, 0:w], KT[:, 2 * t:2 * t + 2, ic, off:off + P],
                                         qv[:, 2 * t:2 * t + 2, qs:qs + w],
                                         start=(t == 0), stop=(t == 3),
                                         perf_mode=_DR)
                    nc.scalar.activation(EXPS[:, kb, :], ps[:, 0:QW],
                                         mybir.ActivationFunctionType.Exp,
                                         scale=escale)
                    if shared:
                        nc.scalar.activation(EXPSB[:, kb, :], ps[:, QW:QW + P],
                                             mybir.ActivationFunctionType.Exp,
                                             scale=escale)
                        nc.scalar.activation(EXPS1[:, kb, :], ps[:, QW + P:],
                                             mybir.ActivationFunctionType.Exp,
                                             scale=escale)
                nc.vector.tensor_mul(EXPS[:, Kb - 4:Kb, :],
                                     EXPS[:, Kb - 4:Kb, :], mt[:])
                rs = ps_rs.tile([1, 2 * QW], _f32)
                for kb in range(Kb):
                    nc.tensor.matmul(rs[0:1, 0:QW], ONES[:, 0, jc:jc + 1],
                                     EXPS[:, kb, :],
                                     start=(kb == 0), stop=(kb == Kb - 1))
                rs1 = spool.tile([1, QW], _f32, tag="rs1")
                nc.vector.tensor_copy(rs1[0:1, :], rs[0:1, 0:QW])
                rsd = dpool.tile([1, QW], _f32)
                nc.sync.dma_start(rsd[:], rs1[0:1, :])
                rst = spool.tile([P, 2], _f32, tag="rst")
                nc.sync.dma_start(
                    rst[:], rsd.rearrange("o (q p) -> (o p) q", p=P, q=2))
                rcp = spool.tile([P, 2], _f32, tag="rcp")
                nc.vector.reciprocal(rcp[:], rst[:])
                for db in range(DB):
                    pst = ps_tt.tile([P, 2 * QW], _f32)
                    for i in range(Kb // 2):
                        nc.tensor.matmul(pst[:, 0:QW], XK[:, db, 2 * i:2 * i + 2, :],
                                         EXPS[:, 2 * i:2 * i + 2, :],
                                         start=(i == 0), stop=(i == Kb // 2 - 1),
                                         perf_mode=_DR)
                    nc.vector.tensor_scalar_mul(TT[:, db, :], pst[:, 0:QW], TTS[jc])
                for qb in range(2):
                    for ec in range(2):
                        po = ps_out.tile([P, NCOL], _f32, tag="po")
                        for t in range(4):
                            nc.tensor.matmul(po[:], TT[:, 2 * t:2 * t + 2, qb * P:(qb + 1) * P],
                                             WV[:, 2 * t:2 * t + 2, ec, :],
                                             start=(t == 0), stop=(t == 3),
                                             perf_mode=_DR)
                        ot = opool.tile([P, NCOL], _f32, tag="ot")
                        nc.vector.tensor_scalar_mul(ot[:], po[:], rcp[:, qb:qb + 1])
                        nc.sync.dma_start(
                            out[qs + qb * P: qs + (qb + 1) * P,
                                ec * NCOL:(ec + 1) * NCOL],
                            ot[:])

            # --- joint cols 2 (ext 4, Kb=16) + 3 (ext 3, Kb=12) ---
            # qv cols [512,1024) are contiguous; shared kbs 0..11 get one
            # 512-wide scores matmul group + one exp (LDWEIGHTS fully hidden
            # behind 512-row matmuls); col2-only kbs 12..15 stay 256-wide.
            # EXPS2[p, kb, col, qn]; col3's slots are its own -> no WAR chain
            # serializing the kernel tail.
            ev2 = EXPS2.rearrange("p k c n -> p k (c n)")
            for kb in range(12):
                ps = ps_sc.tile([P, 2 * QW], _f32)
                ic, off = kb // 4, (kb % 4) * P
                for t in range(4):
                    nc.tensor.matmul(ps[:], KT[:, 2 * t:2 * t + 2, ic, off:off + P],
                                     qv[:, 2 * t:2 * t + 2, 2 * QW:4 * QW],
                                     start=(t == 0), stop=(t == 3),
                                     perf_mode=_DR)
                nc.scalar.activation(ev2[:, kb, :], ps[:],
                                     mybir.ActivationFunctionType.Exp,
                                     scale=escale)
            for kb in range(12, 16):
                ps = ps_sc.tile([P, 2 * QW], _f32)
                ic, off = kb // 4, (kb % 4) * P
                for t in range(4):
                    nc.tensor.matmul(ps[:, 0:QW], KT[:, 2 * t:2 * t + 2, ic, off:off + P],
                                     qv[:, 2 * t:2 * t + 2, 2 * QW:3 * QW],
                                     start=(t == 0), stop=(t == 3),
                                     perf_mode=_DR)
                nc.scalar.activation(EXPS2[:, kb, 0, :], ps[:, 0:QW],
                                     mybir.ActivationFunctionType.Exp,
                                     scale=escale)
            nc.vector.tensor_mul(EXPS2[:, 12:16, 0, :],
                                 EXPS2[:, 12:16, 0, :], mt2[:])
            nc.vector.tensor_mul(EXPS2[:, 8:12, 1, :],
                                 EXPS2[:, 8:12, 1, :], mt3[:])
            # joint rowsum: [1, 512] = [col2-sums | col3-sums]; col2-only kbs
            # accumulate into the low half of the same group
            rs = ps_rs.tile([1, 2 * QW], _f32)
            for kb in range(12):
                nc.tensor.matmul(rs[0:1, :], ONES[:, 0, 2:3],
                                 ev2[:, kb, :],
                                 start=(kb == 0), stop=False)
            for kb in range(12, 16):
                nc.tensor.matmul(rs[0:1, 0:QW], ONES[:, 0, 2:3],
                                 EXPS2[:, kb, 0, :],
                                 start=False, stop=(kb == 15))
            rs1 = spool.tile([1, 2 * QW], _f32, tag="jrs1")
            nc.vector.tensor_copy(rs1[0:1, :], rs[0:1, :])
            rsd = dpool.tile([1, 2 * QW], _f32)
            nc.sync.dma_start(rsd[:], rs1[0:1, :])
            rst = spool.tile([P, 4], _f32, tag="jrst")
            nc.sync.dma_start(
                rst[:], rsd.rearrange("o (c q p) -> (o p) (c q)", p=P, q=2))
            rcp2 = spool.tile([P, 4], _f32, tag="jrcp")
            nc.vector.reciprocal(rcp2[:], rst[:])
            # TT for both cols in one 512-wide psum: shared kb pairs hit both
            # halves, col2-only pairs accumulate into the low half
            for db in range(DB):
                pst = ps_tt.tile([P, 2 * QW], _f32)
                for i in range(6):
                    nc.tensor.matmul(pst[:], XK[:, db, 2 * i:2 * i + 2, :],
                                     ev2[:, 2 * i:2 * i + 2, :],
                                     start=(i == 0), stop=False,
                                     perf_mode=_DR)
                for i in range(6, 8):
                    nc.tensor.matmul(pst[:, 0:QW], XK[:, db, 2 * i:2 * i + 2, :],
                                     EXPS2[:, 2 * i:2 * i + 2, 0, :],
                                     start=False, stop=(i == 7),
                                     perf_mode=_DR)
                nc.vector.tensor_scalar_mul(TT2[:, db, 0, :], pst[:, 0:QW], TTS[2])
                nc.vector.tensor_scalar_mul(TT2[:, db, 1, :], pst[:, QW:], TTS[3])
            for cj, jc in enumerate((2, 3)):
                qs = jc * QW
                for qb in range(2):
                    for ec in range(2):
                        po = ps_out.tile([P, NCOL], _f32, tag="po")
                        for t in range(4):
                            nc.tensor.matmul(po[:], TT2[:, 2 * t:2 * t + 2, cj, qb * P:(qb + 1) * P],
                                             WV[:, 2 * t:2 * t + 2, ec, :],
                                             start=(t == 0), stop=(t == 3),
                                             perf_mode=_DR)
                        ot = opool.tile([P, NCOL], _f32, tag="ot")
                        # ACT is exp-free here; splitting the normalize muls
                        # across DVE+ACT keeps the out pipeline fed
                        if ec == 0:
                            nc.vector.tensor_scalar_mul(
                                ot[:], po[:], rcp2[:, 2 * cj + qb:2 * cj + qb + 1])
                        else:
                            nc.scalar.mul(
                                ot[:], po[:], rcp2[:, 2 * cj + qb:2 * cj + qb + 1])
                        # ACT is exp-free by now; its HWDGE queue halves the
                        # tail's serialized descriptor generation
                        eng = nc.sync if ec == 0 else nc.scalar
                        eng.dma_start(
                            out[qs + qb * P: qs + (qb + 1) * P,
                                ec * NCOL:(ec + 1) * NCOL],
                            ot[:])

    nc.compile()
    _BUILD_CACHE["nc"] = nc
    return nc


def _host_inputs(x, Wq, Wk, Wv):
    # quantize once; all per-core views derive from the same quantized values
    xf = np.asarray(x, np.float32)
    x8 = xf.astype(F8)                                  # [4, 2048, 1024]
    x16 = xf.astype(BF)
    wq2 = np.ascontiguousarray(
        (np.asarray(Wq, np.float32) * 32.0).astype(F8)
        .reshape(DB, P, EB, P).transpose(1, 2, 0, 3))
    wk2 = np.ascontiguousarray(
        (np.asarray(Wk, np.float32) * 32.0).astype(F8)
        .reshape(DB, P, EB, P).transpose(1, 2, 0, 3))
    wv32 = np.asarray(Wv, np.float32) * 32.0
    wv2 = np.ascontiguousarray(
        wv32.astype(F8).reshape(DB, P, 2, NCOL).transpose(1, 0, 2, 3))
    wvb2 = np.ascontiguousarray(
        wv32.astype(BF).reshape(DB, P, 2, NCOL).transpose(1, 0, 2, 3))
    in_maps = []
    p = np.arange(P)[:, None]
    f = np.arange(QW)[None, :]
    for c in range(8):
        b, h = c // 2, c % 2
        gs = QCOLS[h]
        xb = x8[b]                     # [2048, 1024] fp8
        xbt = xb.T                     # [d, n]
        xt_h = np.ascontiguousarray(
            xbt.reshape(DB, P, 4, NCOL).transpose(1, 2, 0, 3))
        qrows = np.concatenate([np.arange(g * QW, (g + 1) * QW) for g in gs])
        xtq_h = np.ascontiguousarray(
            xb[qrows].T.reshape(DB, P, 2, NCOL).transpose(1, 2, 0, 3))
        xk_h = np.ascontiguousarray(
            xb.reshape(16, P, DB, P).transpose(1, 2, 0, 3))   # [P, DB, 16, P]
        xkb_h = np.ascontiguousarray(
            x16[b, :4 * P].reshape(4, P, DB, P).transpose(1, 2, 0, 3))
        m = np.empty((P, 4, 4, QW), dtype=F8)
        for jc, g in enumerate(gs):
            Kb = 4 * EXT[jc]
            for i, kb in enumerate(range(Kb - 4, Kb)):
                m[:, jc, i, :] = ((kb * P + p) <= (g * QW + f)).astype(F8)
        gb = gs[BCOL]
        mb = np.empty((P, 4, QW), dtype=BF)
        for kb in range(4):
            mb[:, kb, :] = ((kb * P + p) <= (gb * QW + f)).astype(BF)
        in_maps.append({
            "xt": xt_h, "xtq": xtq_h, "xk": xk_h, "xkb": xkb_h,
            "wq": wq2, "wk": wk2, "wv": wv2, "wvb": wvb2,
            "msk": m, "mskb": mb,
            "ones": np.tile(np.array([32.0 * s for s in TTS], F8), (P, 2, 1)),
            "onesb": np.full((P, 1), 32.0, BF),
        })
    return in_maps


def kernel(x, Wq, Wk, Wv, _trace=False, _trace_kwargs=None):
    x = np.asarray(x, dtype=np.float32)
    nc = _build()
    in_maps = _host_inputs(x, Wq, Wk, Wv)
    kw = {}
    if _trace:
        kw = {"trace": True, **(_trace_kwargs or {})}
    res = run_bass_kernel_spmd(nc, in_maps, core_ids=list(range(8)), **kw)
    full = np.empty((4, NSEQ, D), dtype=np.float32)
    for c in range(8):
        b, h = c // 2, c % 2
        o = res.results[c]["out"]
        for jc, g in enumerate(QCOLS[h]):
            full[b, g * QW:(g + 1) * QW] = o[jc * QW:(jc + 1) * QW]
    kernel._last_results = res
    return full


# revision 31
# speedup vs baseline: 1.0298x; 1.0298x over previous
"""Causal single-head attention on 8 trn2 NeuronCores — fp8 DoubleRow + bf16 col.

Problem: x [4, 2048, 1024] f32; Wq/Wk/Wv [1024, 1024] f32.
  q,k,v = x@W*; scores = q@k^T (causal masked, scaled 1/sqrt(1024));
  out = softmax(scores) @ v.

Sharding: 8 cores = 4 batches x 2 query-parities. Core c: batch c//2,
parity h=c%2 owns the 256-row query cols {0,3,4,7} (h=0) or {1,2,5,6}
(h=1) -- both parities see causal extents {1,2,3,4} (in 512-key cols),
so one SPMD program fits all cores exactly; per-core causal masks ride
in as data and cover the <=256 keys of block padding per col.

Precision: all matmuls fp8(e4m3) with MatmulPerfMode.DoubleRow
(operands [128,2,N] = two 128-row contraction tiles per instruction,
fp32 PSUM accumulate, ~2x PE throughput), EXCEPT the value path of the
extent-1 query col (queries < 512): those are the only rows with
concentrated attention, where x/Wv quantization noise lands at full
output magnitude (fp8 there measured 4.5e-2 rel err); they use bf16
x/exp/Wv instead (measured 1.3e-2). Scale management (powers of 2,
exact):
  - Wq/Wk/Wv prescaled x32 on host (raw sigma~0.018 sits in fp8's
    subnormal range below 2^-6).  scores' = 1024*s; exp arg scale 2^-15.
  - fp8 TT copied PSUM->SBUF with per-col scale TTS[jc] (keeps empirical
    |TT| max ~240-500 under fp8 max 240; larger scale on short cols so
    1/rowsum-amplified quantization noise stays small).
  - rowsum ones-vector = 32*TTS[jc] (col jc of a [P,4] fp8 tile; 32.0
    for the bf16 col) makes out = po * (1/rowsum_tile) exact.

Col order by extent (2,1,4,3); cols that share causal key-blocks are
fused so their scores run as one 512-query-wide matmul group (hides the
256-cycle DoubleRow LDWEIGHTS behind 512-row matmuls and halves the
instruction count there): cols 0+1 share kbs 0..3 (one group, two exp
reads -- fp8 half for col 0, bf16 half for col 1), cols 2+3 share kbs
0..11 (joint scores/exp/rowsum/TT in [P,512] PSUM; col2-only kbs 12..15
accumulate into the low half).  Each col keeps its own EXPS slots, so
no WAR chain serializes the kernel tail into a HAM re-throttle.
Startup and tail split DMA descriptor generation (~600ns each,
serialized per engine) across the sync and scalar HWDGE queues.

kernel() is self-contained: shards on host, runs via run_bass_kernel_spmd
on cores 0-7, reassembles the full [4, 2048, 1024] output.
"""

import numpy as np
import ml_dtypes
from contextlib import ExitStack

import concourse.bass as bass
import concourse.mybir as mybir
import concourse.tile as tile
from concourse import bacc
from concourse.bass_utils import run_bass_kernel_spmd

P = 128
D = 1024          # d_in == d_out
NSEQ = 2048
NCOL = 512        # projection moving width / key-col unit
QW = 256          # query col width in phase 2
DB = D // P       # 8 d blocks
EB = D // P       # 8 e blocks
EXT = (2, 1, 4, 3)           # causal extent per local q col, in 512-key cols
BCOL = 1                     # the bf16 value-path col (extent 1)
QCOLS = {0: (3, 0, 7, 4), 1: (2, 1, 6, 5)}  # parity -> global 256-q-cols
# per-col fp8 TT copy scale, by extent {1:1/2, 2:1/4, 3:1/8, 4:1/8}
TTS = tuple({1: 0.5, 2: 0.25, 3: 0.125, 4: 0.125}[e] for e in EXT)

_f32 = mybir.dt.float32
_f8 = mybir.dt.float8e4
_bf = mybir.dt.bfloat16
_DR = mybir.MatmulPerfMode.DoubleRow
F8 = ml_dtypes.float8_e4m3
BF = ml_dtypes.bfloat16

_BUILD_CACHE = {}


def _build():
    if "nc" in _BUILD_CACHE:
        return _BUILD_CACHE["nc"]

    nc = bacc.Bacc("TRN2", target_bir_lowering=False, debug=False, num_devices=8)
    # host-pretiled activations; every DMA reads contiguous records
    # xt[p, ic, db, n]   = x^T[db*128+p, ic*512+n]        fp8
    # xtq[p, jc, db, n]  = gathered-q x^T[db*128+p, .]    fp8
    # xk[p, db, kb, m]   = x[kb*128+p, db*128+m]          fp8 (resident)
    # xkb                = same, keys 0..511 only         bf16
    xt = nc.dram_tensor("xt", [P, 4, DB, NCOL], _f8, kind="ExternalInput").ap()
    xtq = nc.dram_tensor("xtq", [P, 2, DB, NCOL], _f8, kind="ExternalInput").ap()
    xk = nc.dram_tensor("xk", [P, DB, 16, P], _f8, kind="ExternalInput").ap()
    xkb = nc.dram_tensor("xkb", [P, DB, 4, P], _bf, kind="ExternalInput").ap()
    # host-prechunked x32-scaled weights
    wq = nc.dram_tensor("wq", [P, EB, DB, P], _f8, kind="ExternalInput").ap()
    wk = nc.dram_tensor("wk", [P, EB, DB, P], _f8, kind="ExternalInput").ap()
    wv = nc.dram_tensor("wv", [P, DB, 2, NCOL], _f8, kind="ExternalInput").ap()
    wvb = nc.dram_tensor("wvb", [P, DB, 2, NCOL], _bf, kind="ExternalInput").ap()
    msk = nc.dram_tensor("msk", [P, 4, 4, QW], _f8, kind="ExternalInput").ap()
    mskb = nc.dram_tensor("mskb", [P, 4, QW], _bf, kind="ExternalInput").ap()
    onesd = nc.dram_tensor("ones", [P, 2, 4], _f8, kind="ExternalInput").ap()
    onesb = nc.dram_tensor("onesb", [P, 1], _bf, kind="ExternalInput").ap()
    out = nc.dram_tensor("out", [1024, D], _f32, kind="ExternalOutput").ap()

    escale = float(2.0 ** -15)   # exp argument scale: 1/(32*sqrt(D)*32*32)

    with tile.TileContext(nc) as tc, ExitStack() as ctx:
        pers = ctx.enter_context(tc.tile_pool(name="pers", bufs=1))
        KT = pers.tile([P, EB, 4, NCOL], _f8)        # 16 KB/part
        QT = pers.tile([P, EB, 2, NCOL], _f8)        # 8
        XK = pers.tile([P, DB, 16, P], _f8)          # 16
        ONES = pers.tile([P, 2, 4], _f8)

        # ---- phase 1: KT / QT projections ----
        with ExitStack() as p1:
            wpool = p1.enter_context(tc.tile_pool(name="wpool", bufs=1))
            WK = wpool.tile([P, EB, DB, P], _f8)     # 8
            WQ = wpool.tile([P, EB, DB, P], _f8)     # 8
            xcol = p1.enter_context(tc.tile_pool(name="xcol", bufs=3))
            ps_proj = p1.enter_context(tc.tile_pool(name="ps_proj", bufs=4, space="PSUM"))

            xcs = [xcol.tile([P, DB, NCOL], _f8, tag="xc", name=f"xc{i}") for i in range(4)]
            # startup: descriptor generation serializes at ~600ns/DMA on the
            # sync engine, so the first two DMAs are exactly what matmul #1
            # needs (WK eb0 pair 0 + xc0 pair 0, 160KB); the rest pace behind
            nc.scalar.dma_start(WK[:, 0, 0:2, :], wk[:, 0, 0:2, :])
            nc.sync.dma_start(xcs[0][:, 0:2, :], xt[:, 0, 0:2, :])
            nc.scalar.dma_start(WK[:, 0, 2:, :], wk[:, 0, 2:, :])
            nc.sync.dma_start(xcs[0][:, 2:4, :], xt[:, 0, 2:4, :])
            nc.sync.dma_start(xcs[0][:, 4:8, :], xt[:, 0, 4:8, :])
            for eb in range(1, 4):
                nc.scalar.dma_start(WK[:, eb, :, :], wk[:, eb, :, :])
            nc.sync.dma_start(xcs[1][:], xt[:, 1, :, :])
            for eb in range(4, EB):
                nc.scalar.dma_start(WK[:, eb, :, :], wk[:, eb, :, :])
            nc.sync.dma_start(ONES[:], onesd)

            for ic in range(4):
                xc = xcs[ic]
                for eb in range(EB):
                    if eb == 5 and ic + 2 < 4:
                        nc.sync.dma_start(xcs[ic + 2][:], xt[:, ic + 2, :, :])
                    if eb == 2 and ic >= 2:
                        for wqe in range(4 * (ic - 2), 4 * (ic - 1)):
                            nc.sync.dma_start(WQ[:, wqe, :, :], wq[:, wqe, :, :])
                    ps = ps_proj.tile([P, NCOL], _f32)
                    for t in range(4):
                        nc.tensor.matmul(ps[:], WK[:, eb, 2 * t:2 * t + 2, :],
                                         xc[:, 2 * t:2 * t + 2, :],
                                         start=(t == 0), stop=(t == 3),
                                         perf_mode=_DR)
                    # ACT copy: keeps DVE free, ACT is otherwise idle here
                    nc.scalar.copy(KT[:, eb, ic, :], ps[:])
            # phase-2 bulk loads queued behind the phase-1 stream
            xqs = []
            for jc in range(2):
                xq = xcol.tile([P, DB, NCOL], _f8, tag="xc", name=f"xq{jc}")
                nc.sync.dma_start(xq[:], xtq[:, jc, :, :])
                xqs.append(xq)
            for db in range(DB):
                nc.sync.dma_start(XK[:, db, :, :], xk[:, db, :, :])
            for jc in range(2):
                xq = xqs[jc]
                for eb in range(EB):
                    ps = ps_proj.tile([P, NCOL], _f32)
                    for t in range(4):
                        nc.tensor.matmul(ps[:], WQ[:, eb, 2 * t:2 * t + 2, :],
                                         xq[:, 2 * t:2 * t + 2, :],
                                         start=(t == 0), stop=(t == 3),
                                         perf_mode=_DR)
                    nc.vector.tensor_copy(QT[:, eb, jc, :], ps[:])

        # ---- phase 2: attention, per 256-wide local q col ----
        with ExitStack() as p2:
            p2sb = p2.enter_context(tc.tile_pool(name="p2sb", bufs=1))
            WV = p2sb.tile([P, DB, 2, NCOL], _f8)        # 8
            EXPS = p2sb.tile([P, 8, QW], _f8)            # col0 (ext 2)
            EXPS2 = p2sb.tile([P, 16, 2, QW], _f8)       # cols 2+3 joint
            TT2 = p2sb.tile([P, DB, 2, QW], _f8)
            TT = p2sb.tile([P, DB, QW], _f8)             # 2
            XKB = p2sb.tile([P, DB, 4, P], _bf)          # 8
            WVB = p2sb.tile([P, DB, 2, NCOL], _bf)       # 16
            EXPSB = p2sb.tile([P, 4, P], _bf)            # bcol qb0 (bf16)
            TTB = p2sb.tile([P, DB, P], _bf)
            EXPS1 = p2sb.tile([P, 4, P], _f8)            # bcol qb1 (fp8)
            TT1 = p2sb.tile([P, DB, P], _f8)
            ONESB = p2sb.tile([P, 1], _bf)
            ps_sc = p2.enter_context(tc.tile_pool(name="ps_sc", bufs=3, space="PSUM"))
            ps_rs = p2.enter_context(tc.tile_pool(name="ps_rs", bufs=1, space="PSUM"))
            ps_tt = p2.enter_context(tc.tile_pool(name="ps_tt", bufs=2, space="PSUM"))
            ps_out = p2.enter_context(tc.tile_pool(name="ps_out", bufs=2, space="PSUM"))
            mpool = p2.enter_context(tc.tile_pool(name="mpool", bufs=5))
            spool = p2.enter_context(tc.tile_pool(name="spool", bufs=2))
            dpool = p2.enter_context(tc.tile_pool(name="dram", bufs=4, space="DRAM"))
            opool = p2.enter_context(tc.tile_pool(name="opool", bufs=4))

            nc.sync.dma_start(WV[:], wv)
            nc.sync.dma_start(XKB[:], xkb)
            # WVB feeds the bcol out stage (~57us): two half-DMAs land on
            # two queues, ahead of the masks (which have ~25us of slack)
            nc.sync.dma_start(WVB[:, :, 0, :], wvb[:, :, 0, :])
            nc.sync.dma_start(WVB[:, :, 1, :], wvb[:, :, 1, :])
            nc.sync.dma_start(ONESB[:], onesb)
            # all masks up front: col boundaries never wait on a mask DMA
            mt0 = mpool.tile([P, 4, QW], _f8, tag="mt0")
            nc.sync.dma_start(mt0[:], msk[:, 0, :, :])
            mtb = mpool.tile([P, 4, P], _bf, tag="mtb")
            nc.sync.dma_start(mtb[:], mskb[:, :, 0:P])
            mt1 = mpool.tile([P, 4, QW], _f8, tag="mt1")
            nc.sync.dma_start(mt1[:], msk[:, BCOL, :, :])
            mt2 = mpool.tile([P, 4, QW], _f8, tag="mt2")
            nc.sync.dma_start(mt2[:], msk[:, 2, :, :])
            mt3 = mpool.tile([P, 4, QW], _f8, tag="mt3")
            nc.sync.dma_start(mt3[:], msk[:, 3, :, :])

            qv = QT.rearrange("p eb c n -> p eb (c n)")
            for jc in range(4):
                Kb = 4 * EXT[jc]     # kn 128-blocks this col
                qs = jc * QW
                if jc == BCOL:
                    # --- extent-1 col, value path split by query half:
                    # qb0 (local q 0..127, the concentrated short rows) bf16;
                    # qb1 (>=129 keys, diffuse) fp8 DoubleRow.  Exps came
                    # from col 0's shared score groups. ---
                    nc.vector.tensor_mul(EXPSB[:], EXPSB[:], mtb[:])
                    nc.vector.tensor_mul(EXPS1[:], EXPS1[:], mt1[:, :, P:])
                    rs = ps_rs.tile([1, 2 * QW], _f32)
                    for kb in range(Kb):
                        nc.tensor.matmul(rs[0:1, 0:P], ONESB[:, :],
                                         EXPSB[:, kb, :],
                                         start=(kb == 0), stop=(kb == Kb - 1))
                    for kb in range(Kb):
                        nc.tensor.matmul(rs[0:1, P:QW], ONES[:, 0, BCOL:BCOL + 1],
                                         EXPS1[:, kb, :],
                                         start=(kb == 0), stop=(kb == Kb - 1))
                    rs1 = spool.tile([1, QW], _f32, tag="rs1")
                    nc.vector.tensor_copy(rs1[0:1, :], rs[0:1, 0:QW])
                    rsd = dpool.tile([1, QW], _f32)
                    nc.sync.dma_start(rsd[:], rs1[0:1, :])
                    rst = spool.tile([P, 2], _f32, tag="rst")
                    nc.sync.dma_start(
                        rst[:], rsd.rearrange("o (q p) -> (o p) q", p=P, q=2))
                    rcp = spool.tile([P, 2], _f32, tag="rcp")
                    nc.vector.reciprocal(rcp[:], rst[:])
                    for db in range(DB):
                        pst = ps_tt.tile([P, 2 * QW], _f32)
                        for kb in range(Kb):
                            nc.tensor.matmul(pst[:, 0:P], XKB[:, db, kb, :],
                                             EXPSB[:, kb, :],
                                             start=(kb == 0), stop=(kb == Kb - 1))
                        for i in range(Kb // 2):
                            nc.tensor.matmul(pst[:, P:QW], XK[:, db, 2 * i:2 * i + 2, :],
                                             EXPS1[:, 2 * i:2 * i + 2, :],
                                             start=(i == 0), stop=(i == Kb // 2 - 1),
                                             perf_mode=_DR)
                        nc.vector.tensor_copy(TTB[:, db, :], pst[:, 0:P])
                        nc.vector.tensor_scalar_mul(TT1[:, db, :], pst[:, P:QW],
                                                    TTS[BCOL])
                    for ec in range(2):
                        po = ps_out.tile([P, NCOL], _f32, tag="po")
                        for db in range(DB):
                            nc.tensor.matmul(po[:], TTB[:, db, :],
                                             WVB[:, db, ec, :],
                                             start=(db == 0), stop=(db == DB - 1))
                        ot = opool.tile([P, NCOL], _f32, tag="ot")
                        nc.vector.tensor_scalar_mul(ot[:], po[:], rcp[:, 0:1])
                        nc.sync.dma_start(
                            out[qs: qs + P, ec * NCOL:(ec + 1) * NCOL], ot[:])
                        po1 = ps_out.tile([P, NCOL], _f32, tag="po")
                        for t in range(4):
                            nc.tensor.matmul(po1[:], TT1[:, 2 * t:2 * t + 2, :],
                                             WV[:, 2 * t:2 * t + 2, ec, :],
                                             start=(t == 0), stop=(t == 3),
                                             perf_mode=_DR)
                        ot1 = opool.tile([P, NCOL], _f32, tag="ot")
                        nc.vector.tensor_scalar_mul(ot1[:], po1[:], rcp[:, 1:2])
                        nc.sync.dma_start(
                            out[qs + P: qs + QW, ec * NCOL:(ec + 1) * NCOL], ot1[:])
                    continue
                if jc >= 2:
                    continue   # cols 2+3 handled jointly below
                # --- col 0 (ext 2) fp8 DoubleRow path; kbs 0..3 are shared
                # with col 1 (qv cols 0..511 contiguous): one 512-wide matmul
                # group, then two exp reads (fp8 half for col 0, bf16 half
                # for col 1's value path) ---
                mt = mt0
                for kb in range(Kb):
                    ps = ps_sc.tile([P, 2 * QW], _f32)
                    ic, off = kb // 4, (kb % 4) * P
                    shared = kb < 4
                    w = 2 * QW if shared else QW
                    for t in range(4):
                        nc.tensor.matmul(ps[:, 0:w], KT[:, 2 * t:2 * t + 2, ic, off:off + P],
                                         qv[:, 2 * t:2 * t + 2, qs:qs + w],
                                         start=(t == 0), stop=(t == 3),
                                         perf_mode=_DR)
                    nc.scalar.activation(EXPS[:, kb, :], ps[:, 0:QW],
                                         mybir.ActivationFunctionType.Exp,
                                         scale=escale)
                    if shared:
                        nc.scalar.activation(EXPSB[:, kb, :], ps[:, QW:QW + P],
                                             mybir.ActivationFunctionType.Exp,
                                             scale=escale)
                        nc.scalar.activation(EXPS1[:, kb, :], ps[:, QW + P:],
                                             mybir.ActivationFunctionType.Exp,
                                             scale=escale)
                nc.vector.tensor_mul(EXPS[:, Kb - 4:Kb, :],
                                     EXPS[:, Kb - 4:Kb, :], mt[:])
                rs = ps_rs.tile([1, 2 * QW], _f32)
                for kb in range(Kb):
                    nc.tensor.matmul(rs[0:1, 0:QW], ONES[:, 0, jc:jc + 1],
                                     EXPS[:, kb, :],
                                     start=(kb == 0), stop=(kb == Kb - 1))
                rs1 = spool.tile([1, QW], _f32, tag="rs1")
                nc.vector.tensor_copy(rs1[0:1, :], rs[0:1, 0:QW])
                rsd = dpool.tile([1, QW], _f32)
                nc.sync.dma_start(rsd[:], rs1[0:1, :])
                rst = spool.tile([P, 2], _f32, tag="rst")
                nc.sync.dma_start(
                    rst[:], rsd.rearrange("o (q p) -> (o p) q", p=P, q=2))
                rcp = spool.tile([P, 2], _f32, tag="rcp")
                nc.vector.reciprocal(rcp[:], rst[:])
                for db in range(DB):
                    pst = ps_tt.tile([P, 2 * QW], _f32)
                    for i in range(Kb // 2):
                        nc.tensor.matmul(pst[:, 0:QW], XK[:, db, 2 * i:2 * i + 2, :],
                                         EXPS[:, 2 * i:2 * i + 2, :],
                                         start=(i == 0), stop=(i == Kb // 2 - 1),
                                         perf_mode=_DR)
                    nc.vector.tensor_scalar_mul(TT[:, db, :], pst[:, 0:QW], TTS[jc])
                for qb in range(2):
                    for ec in range(2):
                        po = ps_out.tile([P, NCOL], _f32, tag="po")
                        for t in range(4):
                            nc.tensor.matmul(po[:], TT[:, 2 * t:2 * t + 2, qb * P:(qb + 1) * P],
                                             WV[:, 2 * t:2 * t + 2, ec, :],
                                             start=(t == 0), stop=(t == 3),
                                             perf_mode=_DR)
                        ot = opool.tile([P, NCOL], _f32, tag="ot")
                        nc.vector.tensor_scalar_mul(ot[:], po[:], rcp[:, qb:qb + 1])
                        nc.sync.dma_start(
                            out[qs + qb * P: qs + (qb + 1) * P,
                                ec * NCOL:(ec + 1) * NCOL],
                            ot[:])

            # --- joint cols 2 (ext 4, Kb=16) + 3 (ext 3, Kb=12) ---
            # qv cols [512,1024) are contiguous; shared kbs 0..11 get one
            # 512-wide scores matmul group + one exp (LDWEIGHTS fully hidden
            # behind 512-row matmuls); col2-only kbs 12..15 stay 256-wide.
            # EXPS2[p, kb, col, qn]; col3's slots are its own -> no WAR chain
            # serializing the kernel tail.
            ev2 = EXPS2.rearrange("p k c n -> p k (c n)")
            for kb in range(12):
                ps = ps_sc.tile([P, 2 * QW], _f32)
                ic, off = kb // 4, (kb % 4) * P
                for t in range(4):
                    nc.tensor.matmul(ps[:], KT[:, 2 * t:2 * t + 2, ic, off:off + P],
                                     qv[:, 2 * t:2 * t + 2, 2 * QW:4 * QW],
                                     start=(t == 0), stop=(t == 3),
                                     perf_mode=_DR)
                nc.scalar.activation(ev2[:, kb, :], ps[:],
                                     mybir.ActivationFunctionType.Exp,
                                     scale=escale)
            for kb in range(12, 16):
                ps = ps_sc.tile([P, 2 * QW], _f32)
                ic, off = kb // 4, (kb % 4) * P
                for t in range(4):
                    nc.tensor.matmul(ps[:, 0:QW], KT[:, 2 * t:2 * t + 2, ic, off:off + P],
                                     qv[:, 2 * t:2 * t + 2, 2 * QW:3 * QW],
                                     start=(t == 0), stop=(t == 3),
                                     perf_mode=_DR)
                nc.scalar.activation(EXPS2[:, kb, 0, :], ps[:, 0:QW],
                                     mybir.ActivationFunctionType.Exp,
                                     scale=escale)
            nc.vector.tensor_mul(EXPS2[:, 12:16, 0, :],
                                 EXPS2[:, 12:16, 0, :], mt2[:])
            nc.vector.tensor_mul(EXPS2[:, 8:12, 1, :],
                                 EXPS2[:, 8:12, 1, :], mt3[:])
            # joint rowsum: [1, 512] = [col2-sums | col3-sums]; col2-only kbs
            # accumulate into the low half of the same group
            rs = ps_rs.tile([1, 2 * QW], _f32)
            for kb in range(12):
                nc.tensor.matmul(rs[0:1, :], ONES[:, 0, 2:3],
                                 ev2[:, kb, :],
                                 start=(kb == 0), stop=False)
            for kb in range(12, 16):
                nc.tensor.matmul(rs[0:1, 0:QW], ONES[:, 0, 2:3],
                                 EXPS2[:, kb, 0, :],
                                 start=False, stop=(kb == 15))
            rs1 = spool.tile([1, 2 * QW], _f32, tag="jrs1")
            nc.vector.tensor_copy(rs1[0:1, :], rs[0:1, :])
            rsd = dpool.tile([1, 2 * QW], _f32)
            nc.sync.dma_start(rsd[:], rs1[0:1, :])
            rst = spool.tile([P, 4], _f32, tag="jrst")
            nc.sync.dma_start(
                rst[:], rsd.rearrange("o (c q p) -> (o p) (c q)", p=P, q=2))
            rcp2 = spool.tile([P, 4], _f32, tag="jrcp")
            nc.vector.reciprocal(rcp2[:], rst[:])
            # TT for both cols in one 512-wide psum: shared kb pairs hit both
            # halves, col2-only pairs accumulate into the low half
            for db in range(DB):
                pst = ps_tt.tile([P, 2 * QW], _f32)
                for i in range(6):
                    nc.tensor.matmul(pst[:], XK[:, db, 2 * i:2 * i + 2, :],
                                     ev2[:, 2 * i:2 * i + 2, :],
                                     start=(i == 0), stop=False,
                                     perf_mode=_DR)
                for i in range(6, 8):
                    nc.tensor.matmul(pst[:, 0:QW], XK[:, db, 2 * i:2 * i + 2, :],
                                     EXPS2[:, 2 * i:2 * i + 2, 0, :],
                                     start=False, stop=(i == 7),
                                     perf_mode=_DR)
                nc.vector.tensor_scalar_mul(TT2[:, db, 0, :], pst[:, 0:QW], TTS[2])
                nc.vector.tensor_scalar_mul(TT2[:, db, 1, :], pst[:, QW:], TTS[3])
            for cj, jc in enumerate((2, 3)):
                qs = jc * QW
                for qb in range(2):
                    for ec in range(2):
                        po = ps_out.tile([P, NCOL], _f32, tag="po")
                        for t in range(4):
                            nc.tensor.matmul(po[:], TT2[:, 2 * t:2 * t + 2, cj, qb * P:(qb + 1) * P],
                                             WV[:, 2 * t:2 * t + 2, ec, :],
                                             start=(t == 0), stop=(t == 3),
                                             perf_mode=_DR)
                        ot = opool.tile([P, NCOL], _f32, tag="ot")
                        # ACT is exp-free here; splitting the normalize muls
                        # across DVE+ACT keeps the out pipeline fed
                        if ec == 0:
                            nc.vector.tensor_scalar_mul(
                                ot[:], po[:], rcp2[:, 2 * cj + qb:2 * cj + qb + 1])
                        else:
                            nc.scalar.mul(
                                ot[:], po[:], rcp2[:, 2 * cj + qb:2 * cj + qb + 1])
                        # ACT is exp-free by now; its HWDGE queue halves the
                        # tail's serialized descriptor generation
                        eng = nc.sync if ec == 0 else nc.scalar
                        eng.dma_start(
                            out[qs + qb * P: qs + (qb + 1) * P,
                                ec * NCOL:(ec + 1) * NCOL],
                            ot[:])

    nc.compile()
    _BUILD_CACHE["nc"] = nc
    return nc


def _host_inputs(x, Wq, Wk, Wv):
    # quantize once; all per-core views derive from the same quantized values
    xf = np.asarray(x, np.float32)
    x8 = xf.astype(F8)                                  # [4, 2048, 1024]
    x16 = xf.astype(BF)
    wq2 = np.ascontiguousarray(
        (np.asarray(Wq, np.float32) * 32.0).astype(F8)
        .reshape(DB, P, EB, P).transpose(1, 2, 0, 3))
    wk2 = np.ascontiguousarray(
        (np.asarray(Wk, np.float32) * 32.0).astype(F8)
        .reshape(DB, P, EB, P).transpose(1, 2, 0, 3))
    wv32 = np.asarray(Wv, np.float32) * 32.0
    wv2 = np.ascontiguousarray(
        wv32.astype(F8).reshape(DB, P, 2, NCOL).transpose(1, 0, 2, 3))
    wvb2 = np.ascontiguousarray(
        wv32.astype(BF).reshape(DB, P, 2, NCOL).transpose(1, 0, 2, 3))
    in_maps = []
    p = np.arange(P)[:, None]
    f = np.arange(QW)[None, :]
    for c in range(8):
        b, h = c // 2, c % 2
        gs = QCOLS[h]
        xb = x8[b]                     # [2048, 1024] fp8
        xbt = xb.T                     # [d, n]
        xt_h = np.ascontiguousarray(
            xbt.reshape(DB, P, 4, NCOL).transpose(1, 2, 0, 3))
        qrows = np.concatenate([np.arange(g * QW, (g + 1) * QW) for g in gs])
        xtq_h = np.ascontiguousarray(
            xb[qrows].T.reshape(DB, P, 2, NCOL).transpose(1, 2, 0, 3))
        xk_h = np.ascontiguousarray(
            xb.reshape(16, P, DB, P).transpose(1, 2, 0, 3))   # [P, DB, 16, P]
        xkb_h = np.ascontiguousarray(
            x16[b, :4 * P].reshape(4, P, DB, P).transpose(1, 2, 0, 3))
        m = np.empty((P, 4, 4, QW), dtype=F8)
        for jc, g in enumerate(gs):
            Kb = 4 * EXT[jc]
            for i, kb in enumerate(range(Kb - 4, Kb)):
                m[:, jc, i, :] = ((kb * P + p) <= (g * QW + f)).astype(F8)
        gb = gs[BCOL]
        mb = np.empty((P, 4, QW), dtype=BF)
        for kb in range(4):
            mb[:, kb, :] = ((kb * P + p) <= (gb * QW + f)).astype(BF)
        in_maps.append({
            "xt": xt_h, "xtq": xtq_h, "xk": xk_h, "xkb": xkb_h,
            "wq": wq2, "wk": wk2, "wv": wv2, "wvb": wvb2,
            "msk": m, "mskb": mb,
            "ones": np.tile(np.array([32.0 * s for s in TTS], F8), (P, 2, 1)),
            "onesb": np.full((P, 1), 32.0, BF),
        })
    return in_maps


def kernel(x, Wq, Wk, Wv, _trace=False, _trace_kwargs=None):
    x = np.asarray(x, dtype=np.float32)
    nc = _build()
    in_maps = _host_inputs(x, Wq, Wk, Wv)
    kw = {}
    if _trace:
        kw = {"trace": True, **(_trace_kwargs or {})}
    res = run_bass_kernel_spmd(nc, in_maps, core_ids=list(range(8)), **kw)
    full = np.empty((4, NSEQ, D), dtype=np.float32)
    for c in range(8):
        b, h = c // 2, c % 2
        o = res.results[c]["out"]
        for jc, g in enumerate(QCOLS[h]):
            full[b, g * QW:(g + 1) * QW] = o[jc * QW:(jc + 1) * QW]
    kernel._last_results = res
    return full
